# revision 15
# baseline (speedup 1.0000x reference)
"""Trainium2 Bass kernel for nn_DYConv_2d (dynamic-kernel CNN, 4 DYConv
stages + triplet attention gate head), data-parallel over batch across 8
NeuronCores.

Strategy:
 - batch 64 -> 8 samples/core; all weights replicated on every core.
 - per-sample 3x3 convs as 9 shifted accumulating matmuls (K=cin, M=cout,
   N=row-chunks of the output plane), float32r matmuls (1 cyc/col, N>=256).
 - per-sample dynamic weights aggregated on VectorE from a [cin, 4*9*cout]
   pre-transposed bank via per-partition-scalar MACs; the attn coefficients
   are broadcast down partitions with a K=1 ones-matmul.
 - training-mode BN: local sums via ScalarE accum_out during PSUM
   eviction, sum-of-squares via one VectorE pass per sample, one tiny
   AllReduce per stage (+ one for the 3 gate BNs).
 - gate head: ZPool comps via VectorE strided reduces / GpSimd partition
   max / ones-matmul channel mean; 7x7 single-output-channel convs as one
   K=98 im2col matmul per sample; the final output only needs the spatial
   mean, so gate application contracts to tiny per-sample reductions.
"""
import numpy as np

import concourse.bass as bass
import concourse.bacc as bacc
import concourse.bass_isa as bass_isa
import concourse.mybir as mybir
import concourse.tile as tile
from concourse.bass_utils import run_bass_kernel_spmd

N_CORES = 8
S = 8  # samples per core
TEMP = 34.0
EPS = 1e-5
FP = mybir.dt.float32
BF = mybir.dt.bfloat16
F32R = mybir.dt.float32r
AF = mybir.ActivationFunctionType
ALU = mybir.AluOpType
AX = mybir.AxisListType

# (cin, cout, pad, Hin, Hout, hid)
STAGES = [
    (100, 60, 1, 48, 48, 26),
    (60, 120, 1, 48, 48, 16),
    (120, 120, 0, 48, 46, 31),
    (120, 64, 0, 46, 44, 31),
]
H4 = 44  # final spatial
NB = 64  # full batch


def _chunks(hout, w):
    rmax = 512 // w
    nch = -(-hout // rmax)
    base, rem = divmod(hout, nch)
    out = []
    y0 = 0
    for i in range(nch):
        r = base + (1 if i < rem else 0)
        out.append((y0, r))
        y0 += r
    return out


def build_nc():
    nc = bacc.Bacc(
        "TRN2",
        target_bir_lowering=False,
        debug=False,
        enable_asserts=True,
        num_devices=N_CORES,
    )
    # ---- DRAM parameters -------------------------------------------------
    xin = nc.dram_tensor("x", [S, 100, 48 * 48], BF, kind="ExternalInput")
    wt_d, wb_d, a1_d, a2_d, a2b_d, bng_d, bnb_d = {}, {}, {}, {}, {}, {}, {}
    for i, (cin, cout, pad, hin, hout, hid) in enumerate(STAGES, 1):
        wt_d[i] = nc.dram_tensor(f"wt{i}", [cin, 36 * cout], BF, kind="ExternalInput")
        wb_d[i] = nc.dram_tensor(f"wb{i}", [4, cout], FP, kind="ExternalInput")
        a1_d[i] = nc.dram_tensor(f"a1w{i}", [cin, hid], FP, kind="ExternalInput")
        a2_d[i] = nc.dram_tensor(f"a2w{i}", [hid, 4], FP, kind="ExternalInput")
        a2b_d[i] = nc.dram_tensor(f"a2b{i}", [S, 4], FP, kind="ExternalInput")
        bng_d[i] = nc.dram_tensor(f"bng{i}", [cout, 1], FP, kind="ExternalInput")
        bnb_d[i] = nc.dram_tensor(f"bnb{i}", [cout, 1], FP, kind="ExternalInput")
    fc3w_d = nc.dram_tensor("fc3w", [100, 64], FP, kind="ExternalInput")
    fc3b_d = nc.dram_tensor("fc3b", [S, 64], FP, kind="ExternalInput")
    gw_d = [nc.dram_tensor(f"gw{g}", [98, 64], BF, kind="ExternalInput")
            for g in range(3)]
    gbn_d = nc.dram_tensor("gbn", [1, 6], FP, kind="ExternalInput")
    ident_d = nc.dram_tensor("ident", [16, 16], FP, kind="ExternalInput")

    x1o = nc.dram_tensor("x1o", [S, 64], FP, kind="ExternalOutput")
    o1o = nc.dram_tensor("o1o", [64, S], FP, kind="ExternalOutput")

    with tile.TileContext(nc) as tc:
        V, A, G = nc.vector, nc.scalar, nc.gpsimd
        from contextlib import ExitStack

        est = ExitStack()
        pact = est.enter_context(tc.tile_pool(name="pact", bufs=1))
        psm = est.enter_context(tc.tile_pool(name="psm", bufs=1))
        pc = est.enter_context(tc.tile_pool(name="pc", bufs=1))
        pdram = est.enter_context(tc.tile_pool(name="pdram", bufs=1, space="DRAM"))
        pwt_cm = tc.tile_pool(name="pwt", bufs=1)
        pwt = pwt_cm.__enter__()
        pz_cm = tc.tile_pool(name="pz", bufs=1)
        pz = pz_cm.__enter__()

        def dma(dst, src):
            nc.sync.dma_start(out=dst, in_=src)

        # ---- constants -------------------------------------------------
        wt_t, wb_t, a1_t, a2_t, a2b_t, bng_t, bnb_t = {}, {}, {}, {}, {}, {}, {}
        for i, (cin, cout, pad, hin, hout, hid) in enumerate(STAGES, 1):
            wt_t[i] = pwt.tile([cin, 36 * cout], BF, tag=f"wt{i}", name=f"wt{i}")
            dma(wt_t[i][:], wt_d[i][:, :])
            wb_t[i] = pc.tile([4, cout], FP, tag=f"wb{i}", name=f"wb{i}")
            dma(wb_t[i][:], wb_d[i][:, :])
            a1_t[i] = pc.tile([cin, hid], FP, tag=f"a1w{i}", name=f"a1w{i}")
            dma(a1_t[i][:], a1_d[i][:, :])
            a2_t[i] = pc.tile([hid, 4], FP, tag=f"a2w{i}", name=f"a2w{i}")
            dma(a2_t[i][:], a2_d[i][:, :])
            a2b_t[i] = pc.tile([S, 4], FP, tag=f"a2b{i}", name=f"a2b{i}")
            dma(a2b_t[i][:], a2b_d[i][:, :])
            bng_t[i] = pc.tile([cout, 1], FP, tag=f"bng{i}", name=f"bng{i}")
            dma(bng_t[i][:], bng_d[i][:, :])
            bnb_t[i] = pc.tile([cout, 1], FP, tag=f"bnb{i}", name=f"bnb{i}")
            dma(bnb_t[i][:], bnb_d[i][:, :])
        fc3w_t = pc.tile([100, 64], FP, tag="fc3w")
        dma(fc3w_t[:], fc3w_d[:, :])
        fc3b_t = pc.tile([S, 64], FP, tag="fc3b")
        dma(fc3b_t[:], fc3b_d[:, :])
        gw_t = []
        for g in range(3):
            tb = pc.tile([98, 64], BF, tag=f"gw{g}", name=f"gw{g}")
            dma(tb[:], gw_d[g][:, :])
            gw_t.append(tb)
        gbn_t = pc.tile([1, 6], FP, tag="gbn")
        dma(gbn_t[:], gbn_d[:, :])
        ident_t = pc.tile([16, 16], FP, tag="ident")
        dma(ident_t[:], ident_d[:, :])
        ones_row = pc.tile([1, 128], FP, tag="ones_row")
        V.memset(ones_row[:], 1.0)
        ones_row_bf = pc.tile([1, 128], BF, tag="ones_row_bf")
        V.memset(ones_row_bf[:], 1.0)
        ones_col = pc.tile([128, 1], BF, tag="ones_col")
        V.memset(ones_col[:], 1.0)
        eps_col = pc.tile([128, 1], FP, tag="eps_col")
        V.memset(eps_col[:], EPS)

        # Every DMA-written tile is allocated ONCE here (virgin SBUF) and
        # explicitly ping-ponged, because Tile's DMA-after-DMA queue
        # watermark bookkeeping on slot reuse is unsound; persistent tiles
        # keep every slot handoff away from DMA-DMA pairs.
        HW4 = H4 * H4
        P1H, P1W = 64 + 6, H4 + 6  # (C, W) plane -> 70 x 50
        P2H, P2W = 64 + 6, H4 + 6  # (C, H) plane (kernel transposed)
        P3H, P3W = H4 + 6, H4 + 6  # 50 x 50
        cp1 = pact.tile([16, P1H * P1W], BF, tag="cp1")
        cp2 = pact.tile([16, P2H * P2W], BF, tag="cp2")
        cp3 = pact.tile([16, P3H * P3W], BF, tag="cp3")
        views = []
        for cp, ph, pw in ((cp1, P1H, P1W), (cp2, P2H, P2W), (cp3, P3H, P3W)):
            cpv = cp[:].rearrange("p (h w) -> p h w", h=ph)
            V.memset(cpv[:, 0:3, :], 0.0)
            V.memset(cpv[:, ph - 3 : ph, :], 0.0)
            V.memset(cpv[:, 3 : ph - 3, 0:3], 0.0)
            V.memset(cpv[:, 3 : ph - 3, pw - 3 : pw], 0.0)
            views.append(cpv)
        cp1v, cp2v, cp3v = views
        rhsAB = [pact.tile([98, 2816], BF, tag=f"im2col{j}", name=f"im2col{j}")
                 for j in range(2)]
        s1ts = [psm.tile([64, H4], BF, tag=f"s1t{j}", name=f"s1t{j}")
                for j in range(2)]
        s2ts = [psm.tile([64, H4], BF, tag=f"s2t{j}", name=f"s2t{j}")
                for j in range(2)]
        s3rows = [psm.tile([1, HW4], BF, tag=f"s3row{j}", name=f"s3row{j}")
                  for j in range(2)]
        m3s4 = [psm.tile([1, 484], BF, tag=f"m3s{j}", name=f"m3s{j}")
                for j in range(4)]
        gflat = psm.tile([1, 48], FP, tag="gflat")
        gtot_in = psm.tile([1, 48], FP, tag="gtot_in")
        af32s = [psm.tile([1, 4 * S], FP, tag=f"af32_{j}", name=f"af32_{j}")
                 for j in range(2)]
        msums = [psm.tile([120, 1], FP, tag=f"msum{j}", name=f"msum{j}")
                 for j in range(2)]
        msqs = [psm.tile([120, 1], FP, tag=f"msq{j}", name=f"msq{j}")
                for j in range(2)]

        stage_ps_cm = tc.tile_pool(name="stageps", bufs=1, space="PSUM")
        stage_ps = stage_ps_cm.__enter__()

        # ---- phase A: load x (padded), pooled1, x1 ---------------------
        cur_tiles = []
        pooledT = psm.tile([100, S], FP, tag="pooled", bufs=2)
        for b in range(S):
            xt = pact.tile([100, 50 * 50], BF, tag="xt", bufs=S)
            xv = xt[:].rearrange("p (h w) -> p h w", h=50)
            V.memset(xv[:, 0, :], 0.0)
            V.memset(xv[:, 49, :], 0.0)
            V.memset(xv[:, 1:49, 0], 0.0)
            V.memset(xv[:, 1:49, 49], 0.0)
            dma(xv[:, 1:49, 1:49], xin[b, :, :].rearrange("p (h w) -> p h w", h=48))
            V.tensor_reduce(
                pooledT[:, b : b + 1], xv[:, 1:49, 1:49], axis=AX.XY, op=ALU.add
            )
            cur_tiles.append(xt)

        ps_x1 = stage_ps.tile([S, 64], FP, tag="smallps", bufs=2)
        nc.tensor.matmul(ps_x1[:], pooledT[:], fc3w_t[:], start=True, stop=True)
        x1sb = psm.tile([S, 64], FP, tag="x1sb")
        V.tensor_tensor(x1sb[:], ps_x1[:], fc3b_t[:], op=ALU.add)
        dma(x1o[:, :], x1sb[:])

        # ---- stages ----------------------------------------------------
        y4 = None
        for i, (cin, cout, pad, hin, hout, hid) in enumerate(STAGES, 1):
            wout = hout
            chunks = _chunks(hout, wout)
            nch = len(chunks)

            # --- attention MLP + softmax (all samples at once) ---
            ps_h = stage_ps.tile([hid, S], FP, tag="smallps", bufs=2)
            nc.tensor.matmul(ps_h[:], a1_t[i][:], pooledT[:cin, :], start=True,
                             stop=True)
            hT = psm.tile([hid, S], FP, tag="hT", bufs=2)
            A.activation(hT[:], ps_h[:], AF.Relu)
            ps_l = stage_ps.tile([S, 4], FP, tag="smallps", bufs=2)
            nc.tensor.matmul(ps_l[:], hT[:], a2_t[i][:], start=True, stop=True)
            lg = psm.tile([S, 4], FP, tag="lg", bufs=2)
            V.tensor_tensor(lg[:], ps_l[:], a2b_t[i][:], op=ALU.add)
            mx = psm.tile([S, 1], FP, tag="mx", bufs=2)
            V.tensor_reduce(mx[:], lg[:], axis=AX.X, op=ALU.max)
            nbt = psm.tile([S, 1], FP, tag="nbt", bufs=2)
            V.tensor_scalar(nbt[:], mx[:], -1.0 / TEMP, None, op0=ALU.mult)
            ex = psm.tile([S, 4], FP, tag="ex", bufs=2)
            es = psm.tile([S, 1], FP, tag="es", bufs=2)
            A.activation(ex[:], lg[:], AF.Exp, bias=nbt[:], scale=1.0 / TEMP,
                         accum_out=es[:])
            rc = psm.tile([S, 1], FP, tag="rc", bufs=2)
            V.reciprocal(rc[:], es[:])
            attn = psm.tile([S, 4], FP, tag="attn", bufs=2)
            V.tensor_scalar(attn[:], ex[:], rc[:], None, op0=ALU.mult)
            ps_aT = stage_ps.tile([4, S], FP, tag="smallps", bufs=2)
            nc.tensor.transpose(ps_aT[:], attn[:], ident_t[0:S, 0:S])
            attnT = psm.tile([4, S], FP, tag="attnT", bufs=2)
            A.activation(attnT[:], ps_aT[:], AF.Copy)
            ps_ab = stage_ps.tile([cout, S], FP, tag="smallps", bufs=2)
            nc.tensor.matmul(ps_ab[:], wb_t[i][:], attnT[:], start=True, stop=True)
            aggbT = psm.tile([cout, S], FP, tag="aggbT", bufs=2)
            A.activation(aggbT[:], ps_ab[:], AF.Copy)
            af32 = af32s[(i - 1) % 2]
            dma(af32[:], attn[:])
            ps_bc = stage_ps.tile([cin, 4 * S], FP, tag="bcastps", bufs=2)
            nc.tensor.matmul(ps_bc[:], ones_row[:, :cin], af32[:], start=True,
                             stop=True)

            sums = psm.tile([cout, S * nch], FP, tag="sums", bufs=2)
            sqs = psm.tile([cout, S], FP, tag="sqs", bufs=2)

            # --- per-sample: aggregate weights, conv, evict, sumsq ---
            wtv = wt_t[i][:].rearrange("p (k t o) -> p k t o", k=4, t=9)
            ztiles = []
            for b in range(S):
                agA = pz.tile([cin, 9 * cout], BF, tag="agA", bufs=2)
                agB = pz.tile([cin, 9 * cout], BF, tag="agB", bufs=2)
                V.tensor_scalar(agA[:], wtv[:, 0, :, :],
                                ps_bc[:, 4 * b : 4 * b + 1], None, op0=ALU.mult)
                V.scalar_tensor_tensor(agB[:], wtv[:, 1, :, :],
                                       ps_bc[:, 4 * b + 1 : 4 * b + 2], agA[:],
                                       op0=ALU.mult, op1=ALU.add)
                V.scalar_tensor_tensor(agA[:], wtv[:, 2, :, :],
                                       ps_bc[:, 4 * b + 2 : 4 * b + 3], agB[:],
                                       op0=ALU.mult, op1=ALU.add)
                V.scalar_tensor_tensor(agB[:], wtv[:, 3, :, :],
                                       ps_bc[:, 4 * b + 3 : 4 * b + 4], agA[:],
                                       op0=ALU.mult, op1=ALU.add)
                agv = agB[:].rearrange("p (t o) -> p t o", t=9)

                zt = pz.tile([cout, hout * wout], BF, tag="zt", bufs=S)
                ztiles.append(zt)
                hview = hin + 2 if pad else hin
                xv = cur_tiles[b][:].rearrange("p (h w) -> p h w", h=hview)
                for ci, (y0, rows) in enumerate(chunks):
                    n = rows * wout
                    ps = stage_ps.tile([cout, 512], FP, tag="convps", bufs=4)
                    for t in range(9):
                        dy, dx = divmod(t, 3)
                        rhs = xv[:, y0 + dy : y0 + dy + rows, dx : dx + wout]
                        nc.tensor.matmul(
                            ps[:, :n],
                            agv[:, t, :],
                            rhs,
                            start=(t == 0),
                            stop=(t == 8),
                        )
                    A.activation(
                        zt[:, y0 * wout : y0 * wout + n],
                        ps[:, :n],
                        AF.Identity,
                        bias=aggbT[:, b : b + 1],
                        accum_out=sums[:, b * nch + ci : b * nch + ci + 1],
                    )
                trash = pact.tile([cout, hout * wout], BF, tag="trash", bufs=1)
                V.scalar_tensor_tensor(trash[:], zt[:], 0.0, zt[:],
                                       op0=ALU.add, op1=ALU.mult,
                                       accum_out=sqs[:, b : b + 1])

            # --- BN stats: local reduce + cross-core all-reduce ---
            stot = psm.tile([cout, 1], FP, tag="stot", bufs=2)
            V.tensor_reduce(stot[:], sums[:], axis=AX.X, op=ALU.add)
            qtot = psm.tile([cout, 1], FP, tag="qtot", bufs=2)
            V.tensor_reduce(qtot[:], sqs[:], axis=AX.X, op=ALU.add)
            bin_t = pdram.tile([2 * cout], FP, tag=f"bnc_in{i}",
                               name=f"bnc_in{i}")
            bout_t = pdram.tile([2 * cout], FP, tag=f"bnc_out{i}",
                                name=f"bnc_out{i}", addr_space="Shared")
            dma(bin_t[0:cout], stot[:])
            dma(bin_t[cout : 2 * cout], qtot[:])
            nc.gpsimd.collective_compute(
                "AllReduce",
                ALU.add,
                ins=[bin_t[:].opt()],
                outs=[bout_t[:].opt()],
                replica_groups=[list(range(N_CORES))],
            )
            msum = msums[(i - 1) % 2]
            dma(msum[:cout, :], bout_t[0:cout])
            msq = msqs[(i - 1) % 2]
            dma(msq[:cout, :], bout_t[cout : 2 * cout])
            ntot = float(NB * hout * wout)
            mean = psm.tile([cout, 1], FP, tag="mean", bufs=2)
            V.tensor_scalar(mean[:], msum[:cout, :], 1.0 / ntot, None, op0=ALU.mult)
            m2t = psm.tile([cout, 1], FP, tag="m2t", bufs=2)
            V.tensor_tensor(m2t[:], mean[:], mean[:], op=ALU.mult)
            var = psm.tile([cout, 1], FP, tag="var", bufs=2)
            V.scalar_tensor_tensor(var[:], msq[:cout, :], 1.0 / ntot, m2t[:],
                                   op0=ALU.mult, op1=ALU.subtract)
            std = psm.tile([cout, 1], FP, tag="std", bufs=2)
            A.activation(std[:], var[:], AF.Sqrt, bias=eps_col[:cout, :])
            rstd = psm.tile([cout, 1], FP, tag="rstd", bufs=2)
            V.reciprocal(rstd[:], std[:])
            gh = psm.tile([cout, 1], FP, tag="gh", bufs=2)
            V.tensor_tensor(gh[:], bng_t[i][:], rstd[:], op=ALU.mult)
            mg = psm.tile([cout, 1], FP, tag="mg", bufs=2)
            V.tensor_tensor(mg[:], mean[:], gh[:], op=ALU.mult)
            bh = psm.tile([cout, 1], FP, tag="bh", bufs=2)
            V.tensor_tensor(bh[:], bnb_t[i][:], mg[:], op=ALU.subtract)

            # --- BN apply + relu -> next stage input (+ pooled accum) ---
            if i < 4:
                pad2 = STAGES[i][2]
                pooledT_next = psm.tile([cout, S], FP, tag="pooled", bufs=2)
                nxt_tiles = []
                for b in range(S):
                    if pad2:
                        xt = pact.tile([cout, (hout + 2) * (wout + 2)], BF,
                                       tag="xt", bufs=S)
                        xv2 = xt[:].rearrange("p (h w) -> p h w", h=hout + 2)
                        V.memset(xv2[:, 0, :], 0.0)
                        V.memset(xv2[:, hout + 1, :], 0.0)
                        V.memset(xv2[:, 1 : hout + 1, 0], 0.0)
                        V.memset(xv2[:, 1 : hout + 1, wout + 1], 0.0)
                        outap = xv2[:, 1 : hout + 1, 1 : wout + 1]
                        inap = ztiles[b][:].rearrange("p (h w) -> p h w", h=hout)
                    else:
                        xt = pact.tile([cout, hout * wout], BF, tag="xt", bufs=S)
                        outap = xt[:]
                        inap = ztiles[b][:]
                    A.activation(outap, inap, AF.Relu, bias=bh[:], scale=gh[:],
                                 accum_out=pooledT_next[:, b : b + 1])
                    nxt_tiles.append(xt)
                cur_tiles = nxt_tiles
                pooledT = pooledT_next
            else:
                y4 = []
                for b in range(S):
                    yt = pact.tile([64, H4 * H4], BF, tag="xt", bufs=S)
                    A.activation(yt[:], ztiles[b][:], AF.Relu, bias=bh[:],
                                 scale=gh[:])
                    y4.append(yt)

        # ================= gate head =================
        # ZPool comps per sample (bf16 outs so they can be DMA'd into the
        # bf16 padded comp planes)
        sum1t, max1t, sum2t, max2t, sum1f, sum2f = [], [], [], [], [], []
        for b in range(S):
            yv = y4[b][:].rearrange("p (h w) -> p h w", h=H4)
            yvT = y4[b][:].rearrange("p (h w) -> p w h", h=H4)
            t1 = psm.tile([64, H4], FP, tag="sum1f", bufs=S)
            V.tensor_reduce(t1[:], yvT, axis=AX.X, op=ALU.add)
            sum1f.append(t1)
            t2 = psm.tile([64, H4], FP, tag="sum2f", bufs=S)
            V.tensor_reduce(t2[:], yv, axis=AX.X, op=ALU.add)
            sum2f.append(t2)
            s1 = psm.tile([64, H4], BF, tag="sum1", bufs=S)
            V.tensor_copy(s1[:], t1[:])
            m1 = psm.tile([64, H4], BF, tag="max1", bufs=S)
            V.tensor_reduce(m1[:], yvT, axis=AX.X, op=ALU.max)
            s2 = psm.tile([64, H4], BF, tag="sum2", bufs=S)
            V.tensor_copy(s2[:], t2[:])
            m2_ = psm.tile([64, H4], BF, tag="max2", bufs=S)
            V.tensor_reduce(m2_[:], yv, axis=AX.X, op=ALU.max)
            sum1t.append(s1)
            max1t.append(m1)
            sum2t.append(s2)
            max2t.append(m2_)

        # gate-3 comps: channel max via gpsimd, channel sum via ones-matmul
        for b in range(S):
            par = pact.tile([64, HW4], BF, tag="par", bufs=1)
            G.partition_all_reduce(par[:], y4[b][:], channels=64,
                                   reduce_op=bass_isa.ReduceOp.max)
            dma(cp3v[2 * b : 2 * b + 1, 3 : 3 + H4, 3 : 3 + H4],
                par[0:1, :].rearrange("p (h w) -> p h w", h=H4))
            for ci in range(4):
                psc = stage_ps.tile([1, 512], FP, tag="convps", bufs=4)
                nc.tensor.matmul(
                    psc[:, :484],
                    ones_col[0:64, :],
                    y4[b][:, ci * 484 : (ci + 1) * 484],
                    start=True, stop=True,
                )
                m3s = m3s4[ci]
                A.activation(m3s[:], psc[0:1, :484], AF.Copy)
                dma(cp3v[2 * b + 1 : 2 * b + 2, 3 + ci * 11 : 3 + ci * 11 + 11,
                         3 : 3 + H4],
                    m3s[:].rearrange("p (h w) -> p h w", h=11))
        # gate-1 comps (plane (C, W)): ch0 = max, ch1 = sums (host-scaled w)
        for b in range(S):
            dma(cp1v[2 * b : 2 * b + 1, 3 : 3 + 64, 3 : 3 + H4], max1t[b][:])
            dma(cp1v[2 * b + 1 : 2 * b + 2, 3 : 3 + 64, 3 : 3 + H4], sum1t[b][:])
        # gate-2 comps: computed on the transposed (C, H) plane with a
        # transposed 7x7 kernel (conv(p.T, w).T == conv(p, w.T)), so the
        # writes and the sigma2 readback need no DMA transposes.
        for b in range(S):
            dma(cp2v[2 * b : 2 * b + 1, 3 : 3 + 64, 3 : 3 + H4], max2t[b][:])
            dma(cp2v[2 * b + 1 : 2 * b + 2, 3 : 3 + 64, 3 : 3 + H4], sum2t[b][:])

        stage_ps_cm.__exit__(None, None, None)

        # --- gate convs: per-sample K=98 im2col matmul ---
        gsums = psm.tile([S, 6], FP, tag="gsums")
        gsb = []
        gate_ps_cm = tc.tile_pool(name="gateps", bufs=1, space="PSUM")
        gate_ps = gate_ps_cm.__enter__()
        gate_geom = ((cp1, P1H, P1W, 64, H4), (cp2, P2H, P2W, 64, H4),
                     (cp3, P3H, P3W, H4, H4))
        for g, (cp, ph, pw, oh, ow) in enumerate(gate_geom):
            plane = oh * ow
            gps = gate_ps.tile([S, plane], FP, tag="gps", bufs=1)
            cpb = cp[:].rearrange("(b c) (h w) -> b c h w", b=S, h=ph)
            nch2 = -(-plane // 512)
            for b in range(S):
                rhs = rhsAB[b % 2]
                for t in range(49):
                    dy, dx = divmod(t, 7)
                    dst = rhs[2 * t : 2 * t + 2, :plane].rearrange(
                        "p (h w) -> p h w", h=oh)
                    dma(dst, cpb[b, :, dy : dy + oh, dx : dx + ow])
                for ci in range(nch2):
                    c0 = ci * 512
                    c1 = min(plane, c0 + 512)
                    # lhsT column b carries the weights, others are zero, so
                    # accumulating over samples writes each sample's conv to
                    # its own psum row.
                    nc.tensor.matmul(
                        gps[:, c0:c1],
                        gw_t[g][:, 8 * b : 8 * b + 8],
                        rhs[:, c0:c1],
                        start=(b == 0), stop=(b == S - 1),
                    )
            sb = psm.tile([S, plane], BF, tag=f"gsb{g}", name=f"gsb{g}")
            A.activation(sb[:], gps[:], AF.Copy,
                         accum_out=gsums[:, 2 * g : 2 * g + 1])
            trash = pact.tile([S, plane], BF, tag="trash", bufs=1)
            V.scalar_tensor_tensor(trash[:], sb[:], 0.0, sb[:],
                                   op0=ALU.add, op1=ALU.mult,
                                   accum_out=gsums[:, 2 * g + 1 : 2 * g + 2])
            gsb.append(sb)
        gate_ps_cm.__exit__(None, None, None)

        # --- one AllReduce for all three gate BNs ---
        dma(gflat[:], gsums[:])
        gbin = pdram.tile([48], FP, tag="gbin")
        gbout = pdram.tile([48], FP, tag="gbout", addr_space="Shared")
        dma(gbin[:], gflat[:])
        nc.gpsimd.collective_compute(
            "AllReduce",
            ALU.add,
            ins=[gbin[:].opt()],
            outs=[gbout[:].opt()],
            replica_groups=[list(range(N_CORES))],
        )
        dma(gtot_in[:], gbout[:])
        gtot = psm.tile([1, 6], FP, tag="gtot")
        V.tensor_reduce(gtot[:], gtot_in[:].rearrange("p (b v) -> p v b", b=8),
                        axis=AX.X, op=ALU.add)
        ghat = psm.tile([1, 3], FP, tag="ghat")
        bhat = psm.tile([1, 3], FP, tag="bhat")
        planes_n = [64 * H4, 64 * H4, H4 * H4]
        for g in range(3):
            n = float(NB * planes_n[g])
            gmean = psm.tile([1, 1], FP, tag="gmean", bufs=3)
            V.tensor_scalar(gmean[:], gtot[:, 2 * g : 2 * g + 1], 1.0 / n, None,
                            op0=ALU.mult)
            gm2 = psm.tile([1, 1], FP, tag="gm2", bufs=3)
            V.tensor_tensor(gm2[:], gmean[:], gmean[:], op=ALU.mult)
            gvar = psm.tile([1, 1], FP, tag="gvar", bufs=3)
            V.scalar_tensor_tensor(gvar[:], gtot[:, 2 * g + 1 : 2 * g + 2],
                                   1.0 / n, gm2[:], op0=ALU.mult,
                                   op1=ALU.subtract)
            gstd = psm.tile([1, 1], FP, tag="gstd", bufs=3)
            A.activation(gstd[:], gvar[:], AF.Sqrt, bias=eps_col[0:1, :])
            grstd = psm.tile([1, 1], FP, tag="grstd", bufs=3)
            V.reciprocal(grstd[:], gstd[:])
            V.tensor_tensor(ghat[:, g : g + 1], gbn_t[0:1, 2 * g : 2 * g + 1],
                            grstd[:], op=ALU.mult)
            gmg = psm.tile([1, 1], FP, tag="gmg", bufs=3)
            V.tensor_tensor(gmg[:], gmean[:], ghat[:, g : g + 1], op=ALU.mult)
            V.tensor_tensor(bhat[:, g : g + 1],
                            gbn_t[0:1, 2 * g + 1 : 2 * g + 2], gmg[:],
                            op=ALU.subtract)
        ghb = psm.tile([S, 3], FP, tag="ghb")
        bhb = psm.tile([S, 3], FP, tag="bhb")
        G.partition_broadcast(ghb[:], ghat[:], channels=S)
        G.partition_broadcast(bhb[:], bhat[:], channels=S)

        sig = []
        for g in range(3):
            A.activation(gsb[g][:], gsb[g][:], AF.Sigmoid,
                         bias=bhb[:, g : g + 1], scale=ghb[:, g : g + 1])
            sig.append(gsb[g])

        # --- contributions: out1 = (c1 + c2 + c3) ---
        c123 = psm.tile([64, 3 * S], FP, tag="c123")
        inv3hw = 1.0 / (3.0 * HW4)
        bc3_ps_cm = tc.tile_pool(name="bc3ps", bufs=1, space="PSUM")
        bc3_ps = bc3_ps_cm.__enter__()
        for b in range(S):
            s1t = s1ts[b % 2]
            dma(s1t[:], sig[0][b : b + 1, :])
            tr1 = psm.tile([64, H4], FP, tag="tr44", bufs=2)
            V.scalar_tensor_tensor(tr1[:], sum1f[b][:], inv3hw, s1t[:],
                                   op0=ALU.mult, op1=ALU.mult,
                                   accum_out=c123[:, b : b + 1])
            s2t = s2ts[b % 2]
            dma(s2t[:], sig[1][b : b + 1, :])
            tr2 = psm.tile([64, H4], FP, tag="tr44b", bufs=2)
            V.scalar_tensor_tensor(tr2[:], sum2f[b][:], inv3hw, s2t[:],
                                   op0=ALU.mult, op1=ALU.mult,
                                   accum_out=c123[:, S + b : S + b + 1])
            s3row = s3rows[b % 2]
            dma(s3row[:], sig[2][b : b + 1, :])
            bc3 = bc3_ps.tile([64, HW4], FP, tag="bc3", bufs=1)
            for ci in range(4):
                c0 = ci * 512
                c1 = min(HW4, c0 + 512)
                nc.tensor.matmul(
                    bc3[:, c0:c1],
                    ones_row_bf[:, 0:64],
                    s3row[:, c0:c1],
                    start=True, stop=True,
                )
            tr3 = pact.tile([64, HW4], BF, tag="trash", bufs=1)
            V.scalar_tensor_tensor(tr3[:], y4[b][:], inv3hw, bc3[:],
                                   op0=ALU.mult, op1=ALU.mult,
                                   accum_out=c123[:, 2 * S + b : 2 * S + b + 1])
        bc3_ps_cm.__exit__(None, None, None)

        o1a = psm.tile([64, S], FP, tag="o1a")
        V.tensor_tensor(o1a[:], c123[:, 0:S], c123[:, S : 2 * S], op=ALU.add)
        o1sb = psm.tile([64, S], FP, tag="o1sb")
        V.tensor_tensor(o1sb[:], o1a[:], c123[:, 2 * S : 3 * S], op=ALU.add)
        dma(o1o[:, :], o1sb[:])

        pz_cm.__exit__(None, None, None)
        pwt_cm.__exit__(None, None, None)
        est.close()

    nc.compile()
    return nc


def prep_in_maps(inputs):
    f32 = np.float32
    bf16 = mybir.dt.np(BF)
    x = np.ascontiguousarray(np.asarray(inputs["x"], f32))
    common = {}
    for i, (cin, cout, pad, hin, hout, hid) in enumerate(STAGES, 1):
        W = np.asarray(inputs[f"d{i}_W"], f32)  # [4,cout,cin,3,3]
        common[f"wt{i}"] = np.ascontiguousarray(
            W.reshape(4, cout, cin, 9).transpose(2, 0, 3, 1).reshape(cin, 36 * cout)
        ).astype(bf16)
        common[f"wb{i}"] = np.ascontiguousarray(np.asarray(inputs[f"d{i}_b"], f32))
        a1 = np.asarray(inputs[f"d{i}_a1w"], f32)
        common[f"a1w{i}"] = np.ascontiguousarray(a1.T / float(hin * hin))
        common[f"a2w{i}"] = np.ascontiguousarray(
            np.asarray(inputs[f"d{i}_a2w"], f32).T)
        common[f"a2b{i}"] = np.ascontiguousarray(
            np.tile(np.asarray(inputs[f"d{i}_a2b"], f32)[None, :], (S, 1)))
        common[f"bng{i}"] = np.ascontiguousarray(
            np.asarray(inputs[f"bn{i}_g"], f32)[:, None])
        common[f"bnb{i}"] = np.ascontiguousarray(
            np.asarray(inputs[f"bn{i}_b"], f32)[:, None])
    common["fc3w"] = np.ascontiguousarray(
        np.asarray(inputs["fc3_w"], f32).T / float(48 * 48))
    common["fc3b"] = np.ascontiguousarray(
        np.tile(np.asarray(inputs["fc3_b"], f32)[None, :], (S, 1)))
    # gate order: (cw: pool over H, len 44), (hc: pool over W, len 44),
    # (hw: pool over C, len 64); mean channel folded into the conv weight.
    for g, (name, plen) in enumerate((("cw", 44.0), ("hc", 44.0), ("hw", 64.0))):
        w = np.asarray(inputs[f"{name}_w"], f32).copy()  # [1,2,7,7]
        if name == "hc":
            w = np.ascontiguousarray(w.transpose(0, 1, 3, 2))
        w[0, 1] /= plen
        wcol = np.ascontiguousarray(w[0].transpose(1, 2, 0)).reshape(98)
        rep = np.zeros((98, 8, 8), f32)
        for b in range(8):
            rep[:, b, b] = wcol
        common[f"gw{g}"] = np.ascontiguousarray(rep.reshape(98, 64)).astype(bf16)
    common["gbn"] = np.ascontiguousarray(np.array(
        [[np.asarray(inputs["cw_g"]).reshape(-1)[0],
          np.asarray(inputs["cw_b"]).reshape(-1)[0],
          np.asarray(inputs["hc_g"]).reshape(-1)[0],
          np.asarray(inputs["hc_b"]).reshape(-1)[0],
          np.asarray(inputs["hw_g"]).reshape(-1)[0],
          np.asarray(inputs["hw_b"]).reshape(-1)[0]]], f32))
    common["ident"] = np.eye(16, dtype=f32)

    in_maps = []
    for c in range(N_CORES):
        m = dict(common)
        m["x"] = np.ascontiguousarray(
            x[c * S : (c + 1) * S].reshape(S, 100, 48 * 48)).astype(bf16)
        in_maps.append(m)
    return in_maps


_NC_CACHE = None
LAST_RESULTS = None


def kernel(**inputs):
    global _NC_CACHE, LAST_RESULTS
    import os

    if _NC_CACHE is None:
        _NC_CACHE = build_nc()
    nc = _NC_CACHE
    in_maps = prep_in_maps(inputs)
    trace = bool(int(os.environ.get("KERNEL_TRACE", "0")))
    res = run_bass_kernel_spmd(
        nc, in_maps, core_ids=list(range(N_CORES)), trace=trace
    )
    LAST_RESULTS = res
    x1 = np.concatenate([res.results[c]["x1o"] for c in range(N_CORES)], axis=0)
    out1 = np.concatenate(
        [res.results[c]["o1o"].T for c in range(N_CORES)], axis=0)
    return x1.astype(np.float32), out1.astype(np.float32)
